# revision 55
# baseline (speedup 1.0000x reference)
"""MLA (low-rank QKV projection + GQA attention) Bass kernel for 8 trn2 cores.

Problem shapes (hardcoded):
  x [B=2, T=2048, D=2048], Wq1 [512,2048], Wq2 [2048,512],
  Wk1/Wv1 [256,2048], Wk2/Wv2 [512,256], Wo [2048,2048]
  HQ=16 q-heads, HKV=4 kv-heads (GROUP=4), DH=128.

v2 (default, zero attn_mask): token-sharded across 8 cores with on-device
collectives, fp16 internals.
  Core c owns tokens [c*512, (c+1)*512) of the flattened [B*T]; cores 0-3
  cover batch 0, cores 4-7 batch 1.
  phase 1: low-rank qkv projections for the core's tokens only (all heads):
    klowT/vlowT [256,512], kT [512,512], v [512,512] (bt-major), qlowT,
    qT [2048,512]; k/v go to a DRAM bounce and are AllGathered within the
    4-core batch group -> full-batch kT/v. k transfers in fp8_e4m3 (halves
    the first, latency-critical collective; ~6e-3 rel err, gate 2e-2) and
    is upcast to fp16 on arrival; v stays fp16 (fp8 v breaches the gate).
  phase 2: per q-head h (software-pipelined, two heads of scores in
    flight): scores^T = kT_g.T @ qT_h (psum [keys,1024] pairs),
    E = exp(scale*S) fp16, sumexp tree on DVE+Pool folded to [128,512],
    ones-row matmul -> sumexp row, fp16 reciprocal, K=1 broadcast matmul
    -> R [128,512], attnT_h = PV * R (normalized, fused into the
    psum->sbuf copy).
  phase 3: outT[dm,bt] = sum_heads WoT_tile^T @ attnT (WoT streamed from
    DRAM, prefetched during the attention tail), fp16 out; host
    transposes/concats the 8 exact row-slices.

v1 (fallback for nonzero attn_mask): head-sharded, folded projections,
partial Wo outputs summed on host (the original kernel).
"""

import os
import sys
import types

import numpy as np

import concourse.bass as bass
import concourse.tile as tile
from concourse import mybir
from concourse import bass_utils


def _ensure_ntff_hook():
    """If BASS_TRACE=1 is set but this axon build lacks antenv.axon_hooks,
    run_bass_kernel_spmd(trace=True) crashes on the import. Recreate the tiny
    get/set module and register the ctypes NTFF hook so tracing degrades
    gracefully (or works) instead. No-op when the real module exists."""
    try:
        import antenv.axon_hooks  # noqa: F401
        return
    except ImportError:
        pass
    try:
        mod = types.ModuleType("antenv.axon_hooks")
        mod._hook = None
        mod.set_axon_ntff_profile_hook = lambda h: setattr(mod, "_hook", h)
        mod.get_axon_ntff_profile_hook = lambda: mod._hook
        sys.modules["antenv.axon_hooks"] = mod
        import antenv

        antenv.axon_hooks = mod
        try:
            from trn_agent_boot.trn_boot import _ntff_profile_via_ctypes

            so = "/opt/axon/libaxon_pjrt.so"
            if os.path.exists(so):
                hook = _ntff_profile_via_ctypes(so)
                if hook is not None:
                    mod.set_axon_ntff_profile_hook(hook)
                    # the NEFF-dir upload needs bucket access this
                    # container doesn't have; keep artifacts local
                    _orig_upload = bass_utils.upload_artifacts

                    def _safe_upload(tmpdir):
                        try:
                            return _orig_upload(tmpdir)
                        except Exception:
                            return tmpdir

                    bass_utils.upload_artifacts = _safe_upload
        except Exception:
            pass
    except Exception:
        pass


_ensure_ntff_hook()

D_MODEL, HQ, HKV, RQ, RKV = 2048, 16, 4, 512, 256
DH = D_MODEL // HQ            # 128
GROUP = HQ // HKV             # 4
B, T = 2, 2048
BT = B * T                    # 4096
NCORES = 8
HPC = HQ // NCORES            # v1: 2 q-heads per core
SBT = BT // NCORES            # v2: 512 tokens per core
SCALE = 1.0 / np.sqrt(DH)
EXP_BIAS = -3.0               # exp(scale*s - 3): keeps fp16 sumexp small

NK = D_MODEL // 128           # 16 contraction tiles over D
NBT = BT // 512               # 8 bt chunks of 512
NTT = BT // 128               # 32 bt tiles of 128
NQC = T // 512                # 4 query chunks per batch
NKK = T // 128                # 16 key tiles per batch

f32 = mybir.dt.float32
f16 = mybir.dt.float16
f8 = mybir.dt.float8e4


class _TC(tile.TileContext):
    pass


_nop_ctr = [0]


def _split_multi_waits(nc):
    """This walrus build's CoreV3 lowering accepts only ONE sync-wait per
    instruction; move extra waits onto same-engine single-wait nops inserted
    immediately before the instruction."""
    for f in nc.m.functions:
        for bb in f.blocks:
            insts = list(bb.instructions)
            out = []
            changed = False
            for ins in insts:
                si = ins.sync_info
                if si is not None and si.on_wait and len(si.on_wait) > 1:
                    waits = list(si.on_wait)
                    for w in waits[:-1]:
                        _nop_ctr[0] += 1
                        nop = mybir.InstNoOp(
                            name=f"waitsplit_{_nop_ctr[0]}",
                            ins=[],
                            outs=[],
                            engine=ins.engine,
                        )
                        nop.sync_info = mybir.SyncInfo(on_wait=[w], on_update=[])
                        nc.register_instruction(nop)
                        out.append(nop)
                    ins.sync_info = mybir.SyncInfo(
                        on_wait=[waits[-1]], on_update=list(si.on_update)
                    )
                    changed = True
                out.append(ins)
            if changed:
                bb.instructions = out


# ======================================================================
# v2: token-sharded, collectives, fp16
# ======================================================================

def _build_v2():
    nc = bass.Bass(trn_type="TRN2")
    xT = nc.dram_tensor("xT", (D_MODEL, SBT), f16, kind="ExternalInput")
    wq1T = nc.dram_tensor("wq1T", (D_MODEL, RQ), f16, kind="ExternalInput")
    wq2T = nc.dram_tensor("wq2T", (RQ, HQ * DH), f16, kind="ExternalInput")
    wk1T = nc.dram_tensor("wk1T", (D_MODEL, RKV), f16, kind="ExternalInput")
    wk2T = nc.dram_tensor("wk2T", (RKV, HKV * DH), f16, kind="ExternalInput")
    wv1T = nc.dram_tensor("wv1T", (D_MODEL, RKV), f16, kind="ExternalInput")
    wv2T = nc.dram_tensor("wv2T", (RKV, HKV * DH), f16, kind="ExternalInput")
    woT = nc.dram_tensor("woT", (D_MODEL, D_MODEL), f16, kind="ExternalInput")
    ones1 = nc.dram_tensor("ones1", (1, 128), f16, kind="ExternalInput")
    outT = nc.dram_tensor("outT", (D_MODEL, SBT), f16, kind="ExternalOutput")

    Exp = mybir.ActivationFunctionType.Exp
    Copy = mybir.ActivationFunctionType.Copy
    KVD = HKV * DH            # 512
    RG = [[0, 1, 2, 3], [4, 5, 6, 7]]

    with _TC(nc) as tc:
        with (
            tc.tile_pool(name="persist", bufs=1) as persist,
            tc.tile_pool(name="consts", bufs=1) as consts,
            tc.tile_pool(name="dramk", bufs=1, space="DRAM") as dramk,
            tc.tile_pool(name="dramv", bufs=1, space="DRAM") as dramv,
            tc.tile_pool(name="dramkg", bufs=1, space="DRAM") as dramkg,
            tc.tile_pool(name="dramvg", bufs=1, space="DRAM") as dramvg,
        ):
            qT_s = persist.tile([128, HQ * SBT], f16)     # head h at cols h*SBT
            # per-group tiles: scores h0 waits only group 0's gather loads
            kT_g = [persist.tile([128, T], f16, name=f"kT_g{g}") for g in range(HKV)]
            v_full = persist.tile([128, NKK * KVD], f16)  # key tile kk at cols kk*KVD
            attnT_s = persist.tile([128, HQ * SBT], f16)  # normalized, head h at h*SBT
            ones1_s = consts.tile([1, 128], f16)
            ones_col = consts.tile([128, 1], f16)
            ebias_s = consts.tile([128, 1], f32)
            nc.sync.dma_start(ones1_s[:], ones1[:])
            nc.sync.dma_start(
                ones_col[:], ones1[:].rearrange("o (p x) -> (o p) x", x=1)
            )
            nc.gpsimd.memset(ebias_s[:], EXP_BIAS)

            kin = dramk.tile([KVD, SBT], f8)
            vin = dramv.tile([SBT, KVD], f16)
            kg = dramkg.tile([4 * KVD, SBT], f8)
            vg = dramvg.tile([4 * SBT, KVD], f16)

            # ------------- phase 1: low-rank qkv projections -------------
            with (
                tc.tile_pool(name="xin", bufs=1) as xin,
                tc.tile_pool(name="low", bufs=1) as low,
                tc.tile_pool(name="kvtmp", bufs=2) as kvtmp,
                tc.tile_pool(name="p1", bufs=4, space="PSUM") as p1,
            ):
                xT_s = xin.tile([128, NK * SBT], f16)
                with tc.tile_pool(name="wkv", bufs=1) as wkv:
                    wk1_s = wkv.tile([128, NK * RKV], f16)
                    wk2_s = wkv.tile([128, 2 * KVD], f16)
                    wv1_s = wkv.tile([128, NK * RKV], f16)
                    wv2_s = wkv.tile([128, 2 * KVD], f16)
                    wq1_s = wkv.tile([128, NK * RQ], f16)
                    wq2_s = wkv.tile([128, 4 * HQ * DH], f16)
                    # interleave per-tile xT/wk1 loads so the first klow
                    # matmul starts after two small transfers, not 3MB
                    for kd in range(NK):
                        nc.sync.dma_start(
                            xT_s[:, kd * SBT : (kd + 1) * SBT],
                            xT[kd * 128 : (kd + 1) * 128, :],
                        )
                        nc.sync.dma_start(
                            wk1_s[:, kd * RKV : (kd + 1) * RKV],
                            wk1T[kd * 128 : (kd + 1) * 128, :],
                        )
                    nc.sync.dma_start(
                        wk2_s[:].rearrange("p (t m) -> p t m", t=2),
                        wk2T[:].rearrange("(t p) m -> p t m", p=128),
                    )
                    klow_s = low.tile([128, 2 * SBT], f16)
                    vlow_s = low.tile([128, 2 * SBT], f16)
                    # klowT [256, SBT]
                    for rt in range(2):
                        ps = p1.tile([128, SBT], f32, tag="ps1")
                        for kd in range(NK):
                            nc.tensor.matmul(
                                ps[:],
                                wk1_s[:, kd * RKV + rt * 128 : kd * RKV + (rt + 1) * 128],
                                xT_s[:, kd * SBT : (kd + 1) * SBT],
                                start=(kd == 0), stop=(kd == NK - 1),
                            )
                        nc.vector.tensor_copy(
                            klow_s[:, rt * SBT : (rt + 1) * SBT], ps[:]
                        )
                    # kT [512 kvd, 512 bt] -> kin
                    ktmp = kvtmp.tile([128, 4 * SBT], f8, tag="ktmp")
                    for kt in range(4):
                        ps = p1.tile([128, SBT], f32, tag="ps1")
                        for rt in range(2):
                            nc.tensor.matmul(
                                ps[:],
                                wk2_s[:, rt * KVD + kt * 128 : rt * KVD + (kt + 1) * 128],
                                klow_s[:, rt * SBT : (rt + 1) * SBT],
                                start=(rt == 0), stop=(rt == 1),
                            )
                        nc.scalar.activation(
                            ktmp[:, kt * SBT : (kt + 1) * SBT], ps[:], Copy
                        )
                        nc.scalar.dma_start(
                            kin[kt * 128 : (kt + 1) * 128, :],
                            ktmp[:, kt * SBT : (kt + 1) * SBT],
                        )
                    # k gather kicks off as early as possible; v compute and
                    # the q projections fill the transfer window
                    nc.gpsimd.collective_compute(
                        "AllGather",
                        mybir.AluOpType.bypass,
                        replica_groups=RG,
                        ins=[kin[:].opt()],
                        outs=[kg[:].opt()],
                    )
                    # wv/wq loads deferred past the k-gather trigger: the kin
                    # stores otherwise queue behind ~6MB of weight loads on the
                    # shared DMA path, delaying the trigger ~30us
                    for kd in range(NK):
                        nc.sync.dma_start(
                            wv1_s[:, kd * RKV : (kd + 1) * RKV],
                            wv1T[kd * 128 : (kd + 1) * 128, :],
                        )
                    nc.sync.dma_start(
                        wv2_s[:].rearrange("p (t m) -> p t m", t=2),
                        wv2T[:].rearrange("(t p) m -> p t m", p=128),
                    )
                    # vlowT [256, SBT]
                    for rt in range(2):
                        ps = p1.tile([128, SBT], f32, tag="ps1")
                        for kd in range(NK):
                            nc.tensor.matmul(
                                ps[:],
                                wv1_s[:, kd * RKV + rt * 128 : kd * RKV + (rt + 1) * 128],
                                xT_s[:, kd * SBT : (kd + 1) * SBT],
                                start=(kd == 0), stop=(kd == NK - 1),
                            )
                        nc.vector.tensor_copy(
                            vlow_s[:, rt * SBT : (rt + 1) * SBT], ps[:]
                        )
                    # v [512 bt, 512 kvd] -> vin
                    vtmp = kvtmp.tile([128, 4 * KVD], f16, tag="vtmp")
                    for btt in range(4):
                        ps = p1.tile([128, KVD], f32, tag="ps1")
                        for rt in range(2):
                            nc.tensor.matmul(
                                ps[:],
                                vlow_s[:, rt * SBT + btt * 128 : rt * SBT + (btt + 1) * 128],
                                wv2_s[:, rt * KVD : (rt + 1) * KVD],
                                start=(rt == 0), stop=(rt == 1),
                            )
                        nc.scalar.activation(
                            vtmp[:, btt * KVD : (btt + 1) * KVD], ps[:], Copy
                        )
                        nc.scalar.dma_start(
                            vin[btt * 128 : (btt + 1) * 128, :],
                            vtmp[:, btt * KVD : (btt + 1) * KVD],
                        )
                    nc.gpsimd.collective_compute(
                        "AllGather",
                        mybir.AluOpType.bypass,
                        replica_groups=RG,
                        ins=[vin[:].opt()],
                        outs=[vg[:].opt()],
                    )
                    nc.sync.dma_start(
                        wq1_s[:].rearrange("p (t m) -> p t m", t=NK),
                        wq1T[:].rearrange("(t p) m -> p t m", p=128),
                    )
                    nc.sync.dma_start(
                        wq2_s[:].rearrange("p (t m) -> p t m", t=4),
                        wq2T[:].rearrange("(t p) m -> p t m", p=128),
                    )

                    # ---- q projections (overlap with the collectives) ----
                    qlow_s = low.tile([128, 4 * SBT], f16)
                    for rt in range(4):
                        ps = p1.tile([128, SBT], f32, tag="ps1")
                        for kd in range(NK):
                            nc.tensor.matmul(
                                ps[:],
                                wq1_s[:, kd * RQ + rt * 128 : kd * RQ + (rt + 1) * 128],
                                xT_s[:, kd * SBT : (kd + 1) * SBT],
                                start=(kd == 0), stop=(kd == NK - 1),
                            )
                        nc.vector.tensor_copy(
                            qlow_s[:, rt * SBT : (rt + 1) * SBT], ps[:]
                        )
                    for qt in range(16):
                        ps = p1.tile([128, SBT], f32, tag="ps1")
                        for rt in range(4):
                            nc.tensor.matmul(
                                ps[:],
                                wq2_s[:, rt * 2048 + qt * 128 : rt * 2048 + (qt + 1) * 128],
                                qlow_s[:, rt * SBT : (rt + 1) * SBT],
                                start=(rt == 0), stop=(rt == 3),
                            )
                        nc.vector.tensor_copy(
                            qT_s[:, qt * SBT : (qt + 1) * SBT], ps[:]
                        )

                # ---- load gathered k/v into SBUF ----
                # kg rows: s*512 + kvd; vg rows: s*512 + bt_local
                # all on the SP queue: the ACT-triggered queue's transfers
                # starve while a collective transfer is in flight
                k8_g = [persist.tile([128, T], f8, name=f"k8_g{g}") for g in range(HKV)]
                for g in range(HKV):
                    for s in range(4):
                        sl = slice(s * SBT, (s + 1) * SBT)
                        nc.sync.dma_start(
                            k8_g[g][:, sl],
                            kg[s * KVD + g * 128 : s * KVD + (g + 1) * 128, :],
                        )
                        nc.vector.tensor_copy(kT_g[g][:, sl], k8_g[g][:, sl])
                for kk in range(NKK):
                    s, off = kk // 4, (kk % 4) * 128
                    nc.sync.dma_start(
                        v_full[:, kk * KVD : (kk + 1) * KVD],
                        vg[s * SBT + off : s * SBT + off + 128, :],
                    )

            # ---------------- phase 2: attention ----------------
            with (
                tc.tile_pool(name="epool", bufs=26) as epool,
                tc.tile_pool(name="supool", bufs=4) as supool,
                tc.tile_pool(name="rpool", bufs=3) as rpool,
                tc.tile_pool(name="wos", bufs=3) as wos,
            ):
                def load_wo(dmt):
                    wo_s = wos.tile([128, 16 * 128], f16, tag="wo", name=f"wo_{dmt}")
                    nc.sync.dma_start(
                        wo_s[:].rearrange("p (t m) -> p t m", t=16),
                        woT[:, dmt * 128 : (dmt + 1) * 128].rearrange(
                            "(t p) m -> p t m", p=128
                        ),
                    )
                    return wo_s

                p2psum = tc.tile_pool(name="stp", bufs=2, space="PSUM")
                stp = p2psum.__enter__()
                pvp_cm = tc.tile_pool(name="pvp", bufs=2, space="PSUM")
                pvp = pvp_cm.__enter__()
                sump_cm = tc.tile_pool(name="sump", bufs=1, space="PSUM")
                sump = sump_cm.__enter__()
                rp_cm = tc.tile_pool(name="rp", bufs=1, space="PSUM")
                rp = rp_cm.__enter__()

                def emit_scores(h):
                    g = h // GROUP
                    es = []
                    # esum tree: DVE carries esA (e0,e2,e4,e6 + merges), Pool
                    # computes two independent pair-sums (e1+e3, e5+e7).
                    esA = supool.tile([128, 1024], f16, tag="esA", name=f"esA_{h}")
                    esB = supool.tile([128, 1024], f16, tag="esB", name=f"esB_{h}")
                    esC = supool.tile([128, 1024], f16, tag="esC", name=f"esC_{h}")
                    for j in range(8):
                        ps = stp.tile([128, 1024], f32, tag="st", name=f"st_{h}_{j}")
                        for half in range(2):
                            kk = 2 * j + half
                            nc.tensor.matmul(
                                ps[:, half * 512 : (half + 1) * 512],
                                kT_g[g][:, kk * 128 : (kk + 1) * 128],
                                qT_s[:, h * SBT : (h + 1) * SBT],
                                start=True, stop=True,
                            )
                        e = epool.tile([128, 1024], f16, tag="e", name=f"e_{h}_{j}")
                        nc.scalar.activation(e[:], ps[:], Exp, scale=SCALE)
                        es.append(e)
                        with nc.allow_low_precision(reason="fp16 sumexp"):
                            if j == 2:
                                nc.vector.tensor_add(esA[:], es[0][:], es[2][:])
                            elif j == 3:
                                nc.gpsimd.tensor_add(esB[:], es[1][:], es[3][:])
                            elif j == 4:
                                nc.vector.tensor_add(esA[:], esA[:], es[4][:])
                            elif j == 6:
                                nc.vector.tensor_add(esA[:], esA[:], es[6][:])
                            elif j == 7:
                                nc.gpsimd.tensor_add(esC[:], es[5][:], es[7][:])
                    esum = supool.tile([128, SBT], f16, tag="esum", name=f"esum_{h}")
                    with nc.allow_low_precision(reason="fp16 sumexp"):
                        nc.vector.tensor_add(esA[:], esA[:], esB[:])
                        nc.vector.tensor_add(esA[:], esA[:], esC[:])
                        # fold the two 512-column halves so the ones-matmul
                        # only processes 512 rows
                        nc.vector.tensor_add(
                            esum[:], esA[:, 0:512], esA[:, 512:1024]
                        )
                    return es, esum

                def emit_pv(h, es):
                    g = h // GROUP
                    ps_pv = pvp.tile([128, SBT], f32, tag="pv", name=f"pv_{h}")
                    for kk in range(NKK):
                        nc.tensor.matmul(
                            ps_pv[:],
                            v_full[:, kk * KVD + g * 128 : kk * KVD + (g + 1) * 128],
                            es[kk // 2][:, (kk % 2) * 512 : (kk % 2) * 512 + 512],
                            start=(kk == 0), stop=(kk == NKK - 1),
                        )
                    return ps_pv

                def emit_sumrecip(h, esum):
                    ps_sum = sump.tile([1, SBT], f32, tag="sum", name=f"sum_{h}")
                    nc.tensor.matmul(
                        ps_sum[:], ones_col[:], esum[:], start=True, stop=True
                    )
                    r16 = rpool.tile([1, SBT], f16, tag="r16", name=f"r16_{h}")
                    with nc.allow_low_precision(reason="fp16 softmax recip"):
                        nc.vector.reciprocal(r16[:], ps_sum[:])
                    return r16

                def emit_norm(h, ps_pv, r16):
                    ps_R = rp.tile([128, SBT], f32, tag="R", name=f"R_{h}")
                    nc.tensor.matmul(ps_R[:], ones1_s[:], r16[:], start=True, stop=True)
                    R_s = rpool.tile([128, SBT], f16, tag="Rs", name=f"Rs_{h}")
                    with nc.allow_low_precision(reason="fp16 softmax normalize"):
                        nc.vector.tensor_copy(R_s[:], ps_R[:])
                        nc.vector.tensor_mul(
                            attnT_s[:, h * SBT : (h + 1) * SBT], ps_pv[:], R_s[:]
                        )

                # software pipeline, two heads of scores in flight. PE
                # program order per iteration:
                #   [PV h] [scores h+2] [ones h] [R h-1]
                # so the PE never waits on the esum tree or the reciprocal,
                # and PV h0 is deferred until two heads of scores are queued
                # (covers the v-gather arriving after the k-gather).
                pipe = [emit_scores(0), emit_scores(1)]
                pend = None
                wo_tiles = []
                for h in range(HQ):
                    es_cur, esum_cur = pipe.pop(0)
                    ps_pv = emit_pv(h, es_cur)
                    if h + 2 < HQ:
                        pipe.append(emit_scores(h + 2))
                    r16 = emit_sumrecip(h, esum_cur)
                    if pend is not None:
                        emit_norm(h - 1, *pend)
                    pend = (ps_pv, r16)
                    if h >= HQ - 3:
                        # prefetch the first WoT column blocks during the
                        # attention tail
                        wo_tiles.append(load_wo(len(wo_tiles)))
                emit_norm(HQ - 1, *pend)
                rp_cm.__exit__(None, None, None)
                sump_cm.__exit__(None, None, None)
                pvp_cm.__exit__(None, None, None)
                p2psum.__exit__(None, None, None)

                # ---------------- phase 3: Wo ----------------
                with (
                    tc.tile_pool(name="og", bufs=3) as og,
                    tc.tile_pool(name="wop", bufs=2, space="PSUM") as wop,
                ):
                    for dmt in range(16):
                        wo_s = wo_tiles[dmt]
                        po = wop.tile([128, SBT], f32, tag="po", name=f"po_{dmt}")
                        for adt in range(16):
                            nc.tensor.matmul(
                                po[:],
                                wo_s[:, adt * 128 : (adt + 1) * 128],
                                attnT_s[:, adt * SBT : (adt + 1) * SBT],
                                start=(adt == 0), stop=(adt == 15),
                            )
                        if dmt + 3 < 16:
                            wo_tiles.append(load_wo(dmt + 3))
                        o_s = og.tile([128, SBT], f16, tag="o", name=f"o_{dmt}")
                        nc.scalar.activation(o_s[:], po[:], Copy)
                        nc.sync.dma_start(
                            outT[dmt * 128 : (dmt + 1) * 128, :], o_s[:]
                        )

    _split_multi_waits(nc)
    return nc


def _prep_inputs_v2(x, Wq1, Wq2, Wk1, Wk2, Wv1, Wv2, Wo):
    xT = np.ascontiguousarray(x.reshape(BT, D_MODEL).T).astype(np.float16)
    wq1T = np.ascontiguousarray(Wq1.T).astype(np.float16)
    wq2T = np.ascontiguousarray(Wq2.T).astype(np.float16)
    wk1T = np.ascontiguousarray(Wk1.T).astype(np.float16)
    wk2T = np.ascontiguousarray(Wk2.T).astype(np.float16)
    wv1T = np.ascontiguousarray(Wv1.T).astype(np.float16)
    wv2T = np.ascontiguousarray(Wv2.T).astype(np.float16)
    woT = np.ascontiguousarray(Wo.T).astype(np.float16)
    ones1 = np.ones((1, 128), np.float16)
    in_maps = []
    for c in range(NCORES):
        in_maps.append({
            "xT": np.ascontiguousarray(xT[:, c * SBT : (c + 1) * SBT]),
            "wq1T": wq1T, "wq2T": wq2T,
            "wk1T": wk1T, "wk2T": wk2T,
            "wv1T": wv1T, "wv2T": wv2T,
            "woT": woT, "ones1": ones1,
        })
    return in_maps


def _run_v2(x, **spmd_kwargs):
    nc = _get_nc_v2()
    in_maps = _prep_inputs_v2(
        x["x"], x["Wq1"], x["Wq2"], x["Wk1"], x["Wk2"], x["Wv1"], x["Wv2"], x["Wo"]
    )
    res = bass_utils.run_bass_kernel_spmd(
        nc, in_maps, core_ids=list(range(NCORES)), **spmd_kwargs
    )
    out = np.empty((BT, D_MODEL), np.float32)
    for c in range(NCORES):
        out[c * SBT : (c + 1) * SBT, :] = res.results[c]["outT"].T.astype(np.float32)
    return out.reshape(B, T, D_MODEL), res


# ======================================================================
# v1: head-sharded fallback (nonzero attn_mask)
# ======================================================================

def _build_v1(mmdt, use_mask):
    nc = bass.Bass(trn_type="TRN2")
    xT = nc.dram_tensor("xT", (D_MODEL, BT), mmdt, kind="ExternalInput")
    wq = nc.dram_tensor("wq", (D_MODEL, HPC * DH), mmdt, kind="ExternalInput")
    wk = nc.dram_tensor("wk", (D_MODEL, DH), mmdt, kind="ExternalInput")
    wv = nc.dram_tensor("wv", (D_MODEL, DH), mmdt, kind="ExternalInput")
    woT = nc.dram_tensor("woT", (HPC * DH, D_MODEL), mmdt, kind="ExternalInput")
    ones = nc.dram_tensor("ones", (128, 1), mmdt, kind="ExternalInput")
    identm = nc.dram_tensor("identm", (128, 128), mmdt, kind="ExternalInput")
    identf = nc.dram_tensor("identf", (128, 128), f32, kind="ExternalInput")
    if use_mask:
        maskT = nc.dram_tensor("maskT", (T, T), f32, kind="ExternalInput")
    out = nc.dram_tensor("out", (BT, D_MODEL), f32, kind="ExternalOutput")

    Exp = mybir.ActivationFunctionType.Exp
    Copy = mybir.ActivationFunctionType.Copy
    with _TC(nc) as tc:
        with (
            tc.tile_pool(name="persist", bufs=1) as persist,
            tc.tile_pool(name="consts", bufs=1) as consts,
        ):
            qT_s = persist.tile([128, HPC * BT], mmdt)
            kT_s = persist.tile([128, BT], mmdt)
            v_s = persist.tile([128, BT], mmdt)
            attnT_s = persist.tile([128, HPC * BT], mmdt)
            recip_s = persist.tile([64, BT], f32)
            rT_s = persist.tile([128, NTT * HPC], f32)
            ones_s = consts.tile([128, 1], mmdt)
            identm_s = consts.tile([128, 128], mmdt)
            identf_s = consts.tile([128, 128], f32)
            nc.sync.dma_start(ones_s[:], ones[:])
            nc.sync.dma_start(identm_s[:], identm[:])
            nc.sync.dma_start(identf_s[:], identf[:])

            with tc.tile_pool(name="vt", bufs=1) as vtp:
                vT_s = vtp.tile([128, BT], mmdt)
                with (
                    tc.tile_pool(name="wgt", bufs=1) as wgt,
                    tc.tile_pool(name="xin", bufs=3) as xin,
                    tc.tile_pool(name="qkvp", bufs=2, space="PSUM") as qkvp,
                ):
                    wq_s = wgt.tile([128, NK * HPC * DH], mmdt)
                    wk_s = wgt.tile([128, NK * DH], mmdt)
                    wv_s = wgt.tile([128, NK * DH], mmdt)
                    nc.sync.dma_start(
                        wq_s[:].rearrange("p (t m) -> p t m", t=NK),
                        wq[:].rearrange("(t p) m -> p t m", p=128),
                    )
                    nc.sync.dma_start(
                        wk_s[:].rearrange("p (t m) -> p t m", t=NK),
                        wk[:].rearrange("(t p) m -> p t m", p=128),
                    )
                    nc.sync.dma_start(
                        wv_s[:].rearrange("p (t m) -> p t m", t=NK),
                        wv[:].rearrange("(t p) m -> p t m", p=128),
                    )
                    for n in range(NBT):
                        ps_q0 = qkvp.tile([128, 512], f32, tag="psq0")
                        ps_q1 = qkvp.tile([128, 512], f32, tag="psq1")
                        ps_k = qkvp.tile([128, 512], f32, tag="psk")
                        ps_v = qkvp.tile([128, 512], f32, tag="psv")
                        for kd in range(NK):
                            xt = xin.tile([128, 512], mmdt, tag="xt")
                            nc.sync.dma_start(
                                xt[:],
                                xT[kd * 128 : (kd + 1) * 128, n * 512 : (n + 1) * 512],
                            )
                            st, sp = kd == 0, kd == NK - 1
                            nc.tensor.matmul(
                                ps_q0[:], wq_s[:, kd * 256 : kd * 256 + 128], xt[:],
                                start=st, stop=sp,
                            )
                            nc.tensor.matmul(
                                ps_q1[:], wq_s[:, kd * 256 + 128 : kd * 256 + 256], xt[:],
                                start=st, stop=sp,
                            )
                            nc.tensor.matmul(
                                ps_k[:], wk_s[:, kd * 128 : (kd + 1) * 128], xt[:],
                                start=st, stop=sp,
                            )
                            nc.tensor.matmul(
                                ps_v[:], wv_s[:, kd * 128 : (kd + 1) * 128], xt[:],
                                start=st, stop=sp,
                            )
                        sl = slice(n * 512, (n + 1) * 512)
                        nc.vector.tensor_copy(qT_s[:, n * 512 : (n + 1) * 512], ps_q0[:])
                        nc.vector.tensor_copy(
                            qT_s[:, BT + n * 512 : BT + (n + 1) * 512], ps_q1[:]
                        )
                        nc.scalar.activation(kT_s[:, sl], ps_k[:], Copy)
                        nc.scalar.activation(vT_s[:, sl], ps_v[:], Copy)

                with tc.tile_pool(name="trp", bufs=4, space="PSUM") as trp:
                    for t in range(NTT):
                        tr = trp.tile([128, 128], mmdt, tag="tr")
                        nc.tensor.transpose(
                            tr[:], vT_s[:, t * 128 : (t + 1) * 128], identm_s[:]
                        )
                        nc.vector.tensor_copy(v_s[:, t * 128 : (t + 1) * 128], tr[:])

            with (
                tc.tile_pool(name="epool", bufs=20) as epool,
                tc.tile_pool(name="mpool", bufs=3) as mpool,
                tc.tile_pool(name="stp", bufs=4, space="PSUM") as stp,
                tc.tile_pool(name="pvp", bufs=2, space="PSUM") as pvp,
                tc.tile_pool(name="sump", bufs=2, space="PSUM") as sump,
            ):
                chunks = [
                    (h, b, qc)
                    for h in range(HPC)
                    for b in range(B)
                    for qc in range(NQC)
                ]

                def emit_scores(ci, kk):
                    h, b, qc = chunks[ci]
                    qsl = qT_s[
                        :,
                        h * BT + b * T + qc * 512 : h * BT + b * T + (qc + 1) * 512,
                    ]
                    ps_st = stp.tile([128, 512], f32, tag="st", name=f"st_{ci}_{kk}")
                    nc.tensor.matmul(
                        ps_st[:],
                        kT_s[:, b * T + kk * 128 : b * T + (kk + 1) * 128],
                        qsl,
                        start=True, stop=True,
                    )
                    if use_mask:
                        mt = mpool.tile([128, 512], f32, tag="mt", name=f"mt_{ci}_{kk}")
                        nc.sync.dma_start(
                            mt[:],
                            maskT[
                                kk * 128 : (kk + 1) * 128,
                                qc * 512 : (qc + 1) * 512,
                            ],
                        )
                        nc.vector.tensor_add(ps_st[:], ps_st[:], mt[:])
                    e = epool.tile([128, 512], mmdt, tag="e", name=f"e_{ci}_{kk}")
                    nc.scalar.activation(e[:], ps_st[:], Exp, scale=SCALE)
                    return e

                es_cur = [emit_scores(0, kk) for kk in range(NKK)]
                for ci in range(len(chunks)):
                    h, b, qc = chunks[ci]
                    ps_pv = pvp.tile([128, 512], f32, tag="pv", name=f"pv_{ci}")
                    ps_sum = sump.tile([1, 512], f32, tag="sum", name=f"sum_{ci}")
                    es_next = []
                    for kk in range(NKK):
                        st, sp = kk == 0, kk == NKK - 1
                        nc.tensor.matmul(
                            ps_pv[:],
                            v_s[:, (b * NKK + kk) * 128 : (b * NKK + kk + 1) * 128],
                            es_cur[kk][:],
                            start=st, stop=sp,
                        )
                        nc.tensor.matmul(
                            ps_sum[:], ones_s[:], es_cur[kk][:],
                            start=st, stop=sp,
                        )
                        if ci + 1 < len(chunks):
                            es_next.append(emit_scores(ci + 1, kk))
                    osl = slice(b * T + qc * 512, b * T + (qc + 1) * 512)
                    nc.vector.reciprocal(
                        recip_s[h * 32 : h * 32 + 1, osl], ps_sum[0:1, :]
                    )
                    nc.scalar.activation(
                        attnT_s[:, h * BT + b * T + qc * 512 : h * BT + b * T + (qc + 1) * 512],
                        ps_pv[:],
                        Copy,
                    )
                    es_cur = es_next

            with tc.tile_pool(name="rtp", bufs=4, space="PSUM") as rtp:
                for i in range(NTT):
                    tr = rtp.tile([128, 64], f32, tag="rtr")
                    nc.tensor.transpose(
                        tr[:],
                        recip_s[0:64, i * 128 : (i + 1) * 128],
                        identf_s[0:64, 0:64],
                    )
                    nc.vector.tensor_copy(rT_s[:, i * HPC : i * HPC + 1], tr[:, 0:1])
                    nc.vector.tensor_copy(
                        rT_s[:, i * HPC + 1 : i * HPC + 2], tr[:, 32:33]
                    )

            with (
                tc.tile_pool(name="wop", bufs=1) as wop,
                tc.tile_pool(name="omg", bufs=4) as omg,
                tc.tile_pool(name="wops", bufs=4, space="PSUM") as wops,
            ):
                woT_s = wop.tile([128, HPC * D_MODEL], mmdt)
                for h in range(HPC):
                    nc.sync.dma_start(
                        woT_s[:, h * D_MODEL : (h + 1) * D_MODEL],
                        woT[h * 128 : (h + 1) * 128, :],
                    )
                for i in range(NTT):
                    for dc in range(4):
                        p0 = wops.tile([128, 512], f32, tag="p0")
                        p1 = wops.tile([128, 512], f32, tag="p1")
                        nc.tensor.matmul(
                            p0[:],
                            attnT_s[:, 0 * BT + i * 128 : 0 * BT + (i + 1) * 128],
                            woT_s[:, 0 * D_MODEL + dc * 512 : 0 * D_MODEL + (dc + 1) * 512],
                            start=True, stop=True,
                        )
                        nc.tensor.matmul(
                            p1[:],
                            attnT_s[:, 1 * BT + i * 128 : 1 * BT + (i + 1) * 128],
                            woT_s[:, 1 * D_MODEL + dc * 512 : 1 * D_MODEL + (dc + 1) * 512],
                            start=True, stop=True,
                        )
                        t0 = omg.tile([128, 512], f32, tag="t0")
                        t1 = omg.tile([128, 512], f32, tag="t1")
                        nc.scalar.activation(
                            t0[:], p0[:], Copy, scale=rT_s[:, i * HPC : i * HPC + 1]
                        )
                        nc.vector.tensor_scalar_mul(
                            t1[:], p1[:], rT_s[:, i * HPC + 1 : i * HPC + 2]
                        )
                        oo = omg.tile([128, 512], f32, tag="oo")
                        nc.vector.tensor_add(oo[:], t0[:], t1[:])
                        nc.sync.dma_start(
                            out[i * 128 : (i + 1) * 128, dc * 512 : (dc + 1) * 512],
                            oo[:],
                        )
    _split_multi_waits(nc)
    return nc


_cache = {}


def _get_nc_v2():
    if "v2" not in _cache:
        _cache["v2"] = _build_v2()
    return _cache["v2"]


def _get_nc_v1(mmdt_name, use_mask):
    key = ("v1", mmdt_name, use_mask)
    if key not in _cache:
        _cache[key] = _build_v1(getattr(mybir.dt, mmdt_name), use_mask)
    return _cache[key]


def _np_dt(mmdt_name):
    if mmdt_name == "bfloat16":
        import ml_dtypes

        return ml_dtypes.bfloat16
    return np.float32


def _prep_inputs_v1(x, attn_mask, Wq1, Wq2, Wk1, Wk2, Wv1, Wv2, Wo, mmdt_name):
    ndt = _np_dt(mmdt_name)
    xT = np.ascontiguousarray(x.reshape(BT, D_MODEL).T).astype(ndt)
    identm = np.eye(128, dtype=np.float32).astype(ndt)
    identf = np.eye(128, dtype=np.float32)
    ones = np.ones((128, 1), np.float32).astype(ndt)
    use_mask = bool(np.any(attn_mask))
    maskT = None
    if use_mask:
        maskT = np.ascontiguousarray(attn_mask[0, 0].T * np.sqrt(DH)).astype(
            np.float32
        )
    Wq1_64, Wq2_64 = Wq1.astype(np.float64), Wq2.astype(np.float64)
    Wk1_64, Wk2_64 = Wk1.astype(np.float64), Wk2.astype(np.float64)
    Wv1_64, Wv2_64 = Wv1.astype(np.float64), Wv2.astype(np.float64)
    in_maps = []
    for c in range(NCORES):
        h0 = c * HPC
        kv = h0 // GROUP
        wq_f = (Wq2_64[h0 * DH : (h0 + HPC) * DH] @ Wq1_64).T
        wk_f = (Wk2_64[kv * DH : (kv + 1) * DH] @ Wk1_64).T
        wv_f = (Wv2_64[kv * DH : (kv + 1) * DH] @ Wv1_64).T
        woT_c = np.ascontiguousarray(Wo[:, h0 * DH : (h0 + HPC) * DH].T)
        m = {
            "xT": xT,
            "wq": np.ascontiguousarray(wq_f).astype(ndt),
            "wk": np.ascontiguousarray(wk_f).astype(ndt),
            "wv": np.ascontiguousarray(wv_f).astype(ndt),
            "woT": woT_c.astype(ndt),
            "ones": ones,
            "identm": identm,
            "identf": identf,
        }
        if use_mask:
            m["maskT"] = maskT
        in_maps.append(m)
    return in_maps, use_mask


def run(x, attn_mask, Wq1, Wq2, Wk1, Wk2, Wv1, Wv2, Wo, **spmd_kwargs):
    use_mask = bool(np.any(attn_mask))
    force_v1 = os.environ.get("BASS_MLA_FORCE_V1") == "1"
    if not use_mask and not force_v1:
        return _run_v2(
            {"x": x, "Wq1": Wq1, "Wq2": Wq2, "Wk1": Wk1, "Wk2": Wk2,
             "Wv1": Wv1, "Wv2": Wv2, "Wo": Wo},
            **spmd_kwargs,
        )
    mmdt_name = os.environ.get("BASS_MLA_DT", "float32r")
    in_maps, use_mask = _prep_inputs_v1(
        x, attn_mask, Wq1, Wq2, Wk1, Wk2, Wv1, Wv2, Wo, mmdt_name
    )
    nc = _get_nc_v1(mmdt_name, use_mask)
    res = bass_utils.run_bass_kernel_spmd(
        nc, in_maps, core_ids=list(range(NCORES)), **spmd_kwargs
    )
    acc = res.results[0]["out"].astype(np.float64)
    for r in res.results[1:]:
        acc += r["out"]
    out = acc.astype(np.float32).reshape(B, T, D_MODEL)
    return out, res


def kernel(x, attn_mask, Wq1, Wq2, Wk1, Wk2, Wv1, Wv2, Wo):
    out, _ = run(x, attn_mask, Wq1, Wq2, Wk1, Wk2, Wv1, Wv2, Wo)
    return out
